# revision 1
# baseline (speedup 1.0000x reference)
"""Trainium2 Bass kernel for nn_MultiHeadAttention_67731634258682.

MHA: B=2, S=8192, D=1024, H=16 heads (depth 64).
Sharding over 8 cores: core c -> (batch b = c//4, head-group g = c%4).
Each core computes its 4 heads end-to-end plus a row-parallel partial of
the output projection; the host sums the 4 partials per batch.

Kernel structure per core (single NEFF, Tile-scheduled):
  P1: QKV projections in transposed layout (QT/KT: [256, S] feature-on-
      partition; V': [S, 4*(64+1)] with ones columns interleaved) -> HBM
      scratch.
  P2: per head-pair stream: resident KT/V' pair in SBUF; per q-tile of 512:
      flash-style k-loop: logits matmul [k=128, q=512] chunks -> PSUM,
      exp on ScalarE (scale=1/8 folded in) PSUM->SBUF in [128, <=1536]
      super-chunks, PV matmul with lhsT=[V_h | ones] (65 cols) accumulating
      U' = [attn_out^T ; denom] in PSUM over the k loop.
      Epilogue: transpose 65x128 blocks on PE, reciprocal + per-partition
      scale on DVE (softmax normalize), transpose back -> OT scratch.
  P3 (interleaved into stream 1): Y[q,:] partial = O^T.T @ Wo_g.
Matmul dtype configurable: f32r (fp32 bits, reduced-precision full-speed
matmul), bf16, or f32 (slow, exact-ish fallback).
"""

import os
import sys
import numpy as np

for _p in ("/opt/trn_rl_repo", "/root/.axon_site/_ro/trn_rl_repo"):
    if os.path.isdir(_p) and _p not in sys.path:
        sys.path.append(_p)

import concourse.bass as bass
import concourse.mybir as mybir
from concourse import bacc, tile
from concourse.bass import ts, ds
from concourse.masks import make_identity
from concourse.bass_utils import run_bass_kernel_spmd

F32 = mybir.dt.float32
BF16 = mybir.dt.bfloat16
F32R = mybir.dt.float32r

B, S, D = 2, 8192, 1024
H = 16
DEPTH = 64          # head dim
G = 4               # head groups (one per core within a batch)
HPG = 4             # heads per group
DG = HPG * DEPTH    # 256 features per group
QT = 512            # q tile
KC = 128            # k chunk (matmul contraction tile)
NDC = D // 128      # 8 contraction chunks for projections

AFT = mybir.ActivationFunctionType


def _chunk_sched(nkc, max_cc=3):
    """[(start_chunk, n_chunks), ...] covering nkc k-chunks."""
    out = []
    c = 0
    while c < nkc:
        cc = min(max_cc, nkc - c)
        out.append((c, cc))
        c += cc
    return out


def build_program(seq=S, mode="bf16", max_cc=2):
    """Build the per-core Bass program. Returns the compiled Bacc object."""
    assert seq % QT == 0
    nqt = seq // QT
    nkc = seq // KC
    sched = _chunk_sched(nkc, max_cc)
    # 512-wide (kc, head) units per logits PSUM slot / exp chunk
    upslot_n = max_cc
    lg_fd = upslot_n * QT

    # dtype of matmul operands (inputs, weights, scratch). float32r is a
    # 4-byte fp32-bits format; walrus requires producers to emit it typed.
    dt_in = {"bf16": BF16, "f32r": F32R}.get(mode, F32)
    dt_mm = dt_in

    def mm(ap):
        return ap

    nc = bacc.Bacc("TRN2", target_bir_lowering=False, debug=False,
                   enable_asserts=False, num_devices=8)

    # ---- external I/O ----
    qT = nc.dram_tensor("qT", [D, seq], dt_in, kind="ExternalInput").ap()
    kT = nc.dram_tensor("kT", [D, seq], dt_in, kind="ExternalInput").ap()
    vT = nc.dram_tensor("vT", [D, seq], dt_in, kind="ExternalInput").ap()
    Wq = nc.dram_tensor("Wq", [D, DG], dt_in, kind="ExternalInput").ap()
    Wk = nc.dram_tensor("Wk", [D, DG], dt_in, kind="ExternalInput").ap()
    Wv = nc.dram_tensor("Wv", [D, DG], dt_in, kind="ExternalInput").ap()
    Wo = nc.dram_tensor("Wo", [DG, D], dt_in, kind="ExternalInput").ap()
    bq = nc.dram_tensor("bq", [DG, 1], F32, kind="ExternalInput").ap()
    bk = nc.dram_tensor("bk", [DG, 1], F32, kind="ExternalInput").ap()
    bv = nc.dram_tensor("bv", [1, DG], dt_in, kind="ExternalInput").ap()
    Y = nc.dram_tensor("Y", [seq, D], F32, kind="ExternalOutput").ap()

    with tile.TileContext(nc) as tc:
        from contextlib import ExitStack
        ctx = ExitStack()
        with ctx:
            const = ctx.enter_context(tc.tile_pool(name="const", bufs=1))
            dram = ctx.enter_context(
                tc.tile_pool(name="dram", bufs=1, space="DRAM"))
            xin = ctx.enter_context(tc.tile_pool(name="xin", bufs=3))
            evac = ctx.enter_context(tc.tile_pool(name="evac", bufs=4))
            res = ctx.enter_context(tc.tile_pool(name="res", bufs=1))
            qtp = ctx.enter_context(tc.tile_pool(name="qtp", bufs=3))
            ppool = ctx.enter_context(tc.tile_pool(name="ppool", bufs=4))
            epi = ctx.enter_context(tc.tile_pool(name="epi", bufs=4))
            otp = ctx.enter_context(tc.tile_pool(name="otp", bufs=3))
            ypool = ctx.enter_context(tc.tile_pool(name="ypool", bufs=3))
            ps_small = ctx.enter_context(
                tc.tile_pool(name="ps_small", bufs=2, space="PSUM"))
            ps_logit = ctx.enter_context(
                tc.tile_pool(name="ps_logit", bufs=2, space="PSUM"))

            # ---- constants ----
            ident = const.tile([128, 128], F32, tag="ident")
            make_identity(nc, ident[:])
            # ones constants: memset an F32 staging tile, DVE-copy into the
            # matmul dtype so the producer "rounds" (required for float32r).
            ones_f32 = const.tile([128, 128], F32, tag="ones_f32")
            nc.any.memset(ones_f32[:], 1.0)
            ones_row = const.tile([1, 128], dt_mm, tag="ones_row")
            nc.vector.tensor_copy(ones_row[:], ones_f32[0:1, :])

            wq_sb = [const.tile([128, DG], dt_in, tag=f"wq{dc}", name=f"wq{dc}")
                     for dc in range(NDC)]
            wk_sb = [const.tile([128, DG], dt_in, tag=f"wk{dc}", name=f"wk{dc}")
                     for dc in range(NDC)]
            wv_sb = [const.tile([128, DG], dt_in, tag=f"wv{dc}", name=f"wv{dc}")
                     for dc in range(NDC)]
            for dc in range(NDC):
                nc.sync.dma_start(wq_sb[dc][:], Wq[ts(dc, 128), :])
                nc.sync.dma_start(wk_sb[dc][:], Wk[ts(dc, 128), :])
                nc.sync.dma_start(wv_sb[dc][:], Wv[ts(dc, 128), :])
            wo_sb = [const.tile([128, D], dt_in, tag=f"wo{i}", name=f"wo{i}") for i in range(2)]
            for i in range(2):
                nc.sync.dma_start(wo_sb[i][:], Wo[ts(i, 128), :])
            bq_sb = [const.tile([128, 1], F32, tag=f"bq{i}", name=f"bq{i}") for i in range(2)]
            bk_sb = [const.tile([128, 1], F32, tag=f"bk{i}", name=f"bk{i}") for i in range(2)]
            for i in range(2):
                nc.sync.dma_start(bq_sb[i][:], bq[ts(i, 128), :])
                nc.sync.dma_start(bk_sb[i][:], bk[ts(i, 128), :])
            bv_sb = const.tile([1, DG], dt_mm, tag="bv_sb")
            nc.sync.dma_start(bv_sb[:], bv[:, :])

            # ---- DRAM scratch ----
            QT_s = dram.tile([DG, seq], dt_mm, tag="QT_s")
            KT_s = dram.tile([DG, seq], dt_mm, tag="KT_s")
            VP_W = HPG * (DEPTH + 1)  # 260: per-head [V_h | ones]
            Vp_s = dram.tile([seq, VP_W], dt_mm, tag="Vp_s")
            OT_s = dram.tile([DG, seq], dt_mm, tag="OT_s")

            nsc = seq // QT

            # ================= P1: projections =================
            # K^T / Q^T: out[feat128, s512] accumulated over 8 d-chunks.
            def qkproj(sc, xsrc, w_sb, b_sb, out_s):
                xt = xin.tile([128, NDC, QT], dt_in, tag="xt", name="xt")
                src = xsrc.rearrange("(c p) s -> p c s", p=128)
                nc.sync.dma_start(xt[:], src[:, :, ts(sc, QT)])
                for f in range(2):
                    ps = ps_small.tile([128, QT], F32, tag="ps_proj",
                                       name="ps")
                    for dc in range(NDC):
                        nc.tensor.matmul(
                            ps[:], mm(w_sb[dc][:, ts(f, 128)]),
                            mm(xt[:, dc, :]),
                            start=(dc == 0), stop=(dc == NDC - 1))
                    ot = evac.tile([128, QT], dt_mm, tag="qk_evac", name="ot")
                    nc.vector.tensor_scalar_add(ot[:], ps[:], b_sb[f][:])
                    nc.scalar.dma_start(out_s[ts(f, 128), ts(sc, QT)], ot[:])

            for sc in range(nsc):
                qkproj(sc, kT, wk_sb, bk_sb, KT_s)

            # V': out[s128, 260] = x_v @ Wv + bv, with ones at col 64 of
            # each per-head 65-col block.
            for sc in range(nsc):
                xt = xin.tile([128, NDC, QT], dt_in, tag="xt")
                src = vT.rearrange("(c p) s -> p c s", p=128)
                nc.sync.dma_start(xt[:], src[:, :, ts(sc, QT)])
                for sub in range(QT // 128):
                    ps = ps_small.tile([128, DG], F32, tag="ps_proj")
                    for dc in range(NDC):
                        nc.tensor.matmul(
                            ps[:], mm(xt[:, dc, ts(sub, 128)]), mm(wv_sb[dc][:]),
                            start=(dc == 0), stop=False)
                    nc.tensor.matmul(ps[:], mm(ones_row[:]), mm(bv_sb[:]),
                                     start=False, stop=True)
                    vsb = evac.tile([128, VP_W], dt_mm, tag="v_evac")
                    vdst = vsb[:].rearrange("p (h x) -> p h x", x=DEPTH + 1)
                    vsrc = ps[:].rearrange("p (h x) -> p h x", x=DEPTH)
                    nc.vector.tensor_copy(vdst[:, :, 0:DEPTH], vsrc)
                    ones_src = ones_f32[:].rearrange("p (h x) -> p h x", x=1)
                    nc.vector.tensor_copy(
                        vdst[:, :, DEPTH:DEPTH + 1], ones_src[:, 0:HPG, :])
                    nc.scalar.dma_start(
                        Vp_s[ds(sc * QT + sub * 128, 128), :], vsb[:])

            # Q^T projection is interleaved into the pair-0 q-tile loop
            # below; emit only the first s-chunk up front.
            qkproj(0, qT, wq_sb, bq_sb, QT_s)

            # ================= P2: attention =================
            scale = 1.0 / float(np.sqrt(DEPTH))

            def epi_rest(pair, qt, u_sbs):
                """Deferred epilogue: normalize + transpose to OT layout,
                then (pair 1) the output projection for this q tile. Emitted
                one q-tile late so its cross-engine chain overlaps the next
                k-loop instead of stalling ACT."""
                ot_acc = otp.tile([128, QT], dt_mm, tag="ot_acc")
                for h in range(2):
                    u_sb = u_sbs[h]
                    for blk in range(QT // 128):
                        t1 = ps_small.tile([128, DEPTH + 1], F32,
                                           tag="ps_proj")
                        nc.tensor.transpose(
                            t1[:], u_sb[:, ts(blk, 128)],
                            ident[0:DEPTH + 1, 0:DEPTH + 1])
                        rq = epi.tile([128, 1], F32, tag="rq")
                        nc.vector.reciprocal(rq[:], t1[:, DEPTH:DEPTH + 1])
                        oq = epi.tile([128, DEPTH], F32, tag="oq")
                        nc.vector.tensor_scalar_mul(
                            oq[:], t1[:, 0:DEPTH], rq[:])
                        t2 = ps_small.tile([DEPTH, 128], F32, tag="ps_proj")
                        nc.tensor.transpose(t2[:], oq[:], ident[:])
                        nc.vector.tensor_copy(
                            ot_acc[ds(DEPTH * h, DEPTH), ts(blk, 128)],
                            t2[:])
                nc.sync.dma_start(OT_s[ts(pair, 128), ts(qt, QT)], ot_acc[:])
                if pair == 1:
                    ota = otp.tile([128, QT], dt_mm, tag="ota")
                    nc.gpsimd.dma_start(ota[:], OT_s[ts(0, 128), ts(qt, QT)])
                    for qs in range(QT // 128):
                        for f in range(2):
                            yp = ps_small.tile([128, 512], F32,
                                               tag="ps_proj")
                            nc.tensor.matmul(
                                yp[:], mm(ota[:, ts(qs, 128)]),
                                mm(wo_sb[0][:, ts(f, 512)]),
                                start=True, stop=False)
                            nc.tensor.matmul(
                                yp[:], mm(ot_acc[:, ts(qs, 128)]),
                                mm(wo_sb[1][:, ts(f, 512)]),
                                start=False, stop=True)
                            ysb = ypool.tile([128, 512], F32, tag="ysb")
                            nc.vector.tensor_copy(ysb[:], yp[:])
                            nc.sync.dma_start(
                                Y[ds(qt * QT + qs * 128, 128), ts(f, 512)],
                                ysb[:])

            res_bufs = 2 if mode == "bf16" else 1
            pending = None
            for pair in range(2):
                kt_res = res.tile([128, seq], dt_mm, tag="kt_res",
                                  bufs=res_bufs)
                for sc in range(nsc):
                    nc.gpsimd.dma_start(kt_res[:, ts(sc, QT)],
                                        KT_s[ts(pair, 128), ts(sc, QT)])
                vp_res = res.tile([128, nkc, 2 * (DEPTH + 1)], dt_mm,
                                  tag="vp_res", bufs=res_bufs)
                for kc_i in range(nkc):
                    nc.gpsimd.dma_start(
                        vp_res[:, kc_i, :],
                        Vp_s[ds(kc_i * 128, 128),
                             ds(pair * 2 * (DEPTH + 1), 2 * (DEPTH + 1))])

                for qt in range(nqt):
                    if pair == 0 and qt + 1 < nsc:
                        qkproj(qt + 1, qT, wq_sb, bq_sb, QT_s)
                    qtt = qtp.tile([128, QT], dt_mm, tag="qtt")
                    nc.gpsimd.dma_start(qtt[:],
                                        QT_s[ts(pair, 128), ts(qt, QT)])
                    pv_ps = [ps_small.tile([DEPTH + 1, QT], F32,
                                           tag=f"pv{h}", bufs=1,
                                           name=f"pv{h}")
                             for h in range(2)]
                    # unit stream: (k-chunk, head) pairs, heads interleaved so
                    # the two K=64 logits matmuls row-pack into the PE array
                    # concurrently (array rows 0-63 / 64-127), sharing one
                    # PSUM slot and one exp per slot.
                    units = [(kc_i, h) for kc_i in range(nkc)
                             for h in range(2)]
                    for u0 in range(0, len(units), upslot_n):
                        su = units[u0:u0 + upslot_n]
                        nfd = len(su) * QT
                        lg = ps_logit.tile([128, lg_fd], F32, tag="lg")
                        for i, (kc_i, h) in enumerate(su):
                            nc.tensor.matmul(
                                lg[:, ts(i, QT)],
                                mm(kt_res[ds(DEPTH * h, DEPTH),
                                          ts(kc_i, KC)]),
                                mm(qtt[ds(DEPTH * h, DEPTH), :]),
                                start=True, stop=True,
                                tile_position=(DEPTH * h, 0))
                        pt = ppool.tile([128, lg_fd], dt_mm, tag="pt")
                        nc.scalar.activation(
                            pt[:, 0:nfd], lg[:, 0:nfd], AFT.Exp, scale=scale)
                        for i, (kc_i, h) in enumerate(su):
                            nc.tensor.matmul(
                                pv_ps[h][:],
                                mm(vp_res[:, kc_i,
                                          ds(h * (DEPTH + 1), DEPTH + 1)]),
                                mm(pt[:, ts(i, QT)]),
                                start=(kc_i == 0), stop=(kc_i == nkc - 1))

                    # spill U' out of PSUM (frees pv banks), defer the rest
                    u_sbs = []
                    for h in range(2):
                        u_sb = epi.tile([DEPTH + 1, QT], F32, tag="u_sb",
                                        bufs=4)
                        nc.vector.tensor_copy(u_sb[:], pv_ps[h][:])
                        u_sbs.append(u_sb)
                    if pending is not None:
                        epi_rest(*pending)
                    pending = (pair, qt, u_sbs)
            epi_rest(*pending)
    nc.compile()
    return nc


_NC_CACHE = {}


def _get_program(seq, mode, max_cc):
    key = (seq, mode, max_cc)
    if key not in _NC_CACHE:
        _NC_CACHE[key] = build_program(seq, mode, max_cc)
    return _NC_CACHE[key]


def make_in_maps(inputs, seq=S, mode="f32r"):
    """Host-side sharding: per-core input dicts."""
    dt = np.dtype("bfloat16") if mode == "bf16" else np.float32
    try:
        import ml_dtypes
        bf16 = ml_dtypes.bfloat16
    except ImportError:
        bf16 = None
    def cast(x):
        if mode == "bf16":
            return x.astype(bf16)
        return x.astype(np.float32)

    q = np.asarray(inputs["q"], np.float32)
    k = np.asarray(inputs["k"], np.float32)
    v = np.asarray(inputs["v"], np.float32)
    Wq = np.asarray(inputs["Wq"], np.float32)
    Wk = np.asarray(inputs["Wk"], np.float32)
    Wv = np.asarray(inputs["Wv"], np.float32)
    Wo = np.asarray(inputs["Wo"], np.float32)
    bq = np.asarray(inputs["bq"], np.float32)
    bk = np.asarray(inputs["bk"], np.float32)
    bv = np.asarray(inputs["bv"], np.float32)

    qTb = [np.ascontiguousarray(q[b].T) for b in range(B)]
    kTb = [np.ascontiguousarray(k[b].T) for b in range(B)]
    vTb = [np.ascontiguousarray(v[b].T) for b in range(B)]

    in_maps = []
    for c in range(8):
        b, g = c // G, c % G
        cols = slice(g * DG, (g + 1) * DG)
        in_maps.append({
            "qT": cast(qTb[b]), "kT": cast(kTb[b]), "vT": cast(vTb[b]),
            "Wq": cast(np.ascontiguousarray(Wq[:, cols])),
            "Wk": cast(np.ascontiguousarray(Wk[:, cols])),
            "Wv": cast(np.ascontiguousarray(Wv[:, cols])),
            "Wo": cast(np.ascontiguousarray(Wo[cols, :])),
            "bq": np.ascontiguousarray(bq[cols].reshape(DG, 1)),
            "bk": np.ascontiguousarray(bk[cols].reshape(DG, 1)),
            "bv": cast(np.ascontiguousarray(bv[cols].reshape(1, DG))),
        })
    return in_maps


LAST_RESULT = None


def kernel(**inputs):
    global LAST_RESULT
    mode = os.environ.get("MHA_MODE", "bf16")
    max_cc = int(os.environ.get("MHA_MAX_CC", "2"))
    nc = _get_program(S, mode, max_cc)
    in_maps = make_in_maps(inputs, S, mode)
    res = run_bass_kernel_spmd(nc, in_maps, list(range(8)))
    LAST_RESULT = res
    bo = np.asarray(inputs["bo"], np.float32)
    out = np.zeros((B, S, D), np.float32)
    for c in range(8):
        b = c // G
        out[b] += res.results[c]["Y"]
    out += bo[None, None, :]
    return out


if __name__ == "__main__":
    # smoke build
    nc = build_program(512, "f32r")
    print("built ok")



# revision 5
# speedup vs baseline: 1.4259x; 1.4259x over previous
"""Trainium2 Bass kernel for nn_MultiHeadAttention_67731634258682.

MHA: B=2, S=8192, D=1024, H=16 heads (depth 64).
Sharding over 8 cores: core c -> (batch b = c//4, head-group g = c%4).
Each core computes its 4 heads end-to-end plus a row-parallel partial of
the output projection; the host sums the 4 partials per batch.

v2 design (vs v1 baseline at 3.09 ms):
  - Everything SBUF-resident: K^T/Q^T (2 pairs x [128, S]), V' (2 pairs x
    [128, nkc, 130] with ones columns), pair-0 O^T [128, S]. Projections
    evacuate straight into the resident tiles (no DRAM scratch round-trip).
  - Exp split across engines: ~2/3 of (kc) slots on ScalarE (exact Exp,
    scale folded), ~1/3 on DVE via a Schraudolph bit-trick: bf16 bits =
    int16(round(A*logit + B)), one fused tensor_scalar per slot. Rel-err
    cost measured in simulation: 6.0e-3 -> 1.2e-2 (gate 2e-2).
  - Software-pipelined emission: per slot emit exp(i), lg(i+2), pv(i) so
    the in-order PE stream never head-of-line blocks on an exp; epilogue /
    Wo / next-tile q-projection are chopped into small chunks emitted
    between slots to keep the PE continuously busy (p-state ramp to 2.4
    GHz requires gapless execution).
  - DMA issue moved off ScalarE entirely (sync for loads, gpsimd for Y).
"""

import os
import sys
import numpy as np

for _p in ("/opt/trn_rl_repo", "/root/.axon_site/_ro/trn_rl_repo"):
    if os.path.isdir(_p) and _p not in sys.path:
        sys.path.append(_p)

import concourse.bass as bass
import concourse.mybir as mybir
from concourse import bacc, tile
from concourse.bass import ts, ds
from concourse.masks import make_identity
from concourse.bass_utils import run_bass_kernel_spmd

F32 = mybir.dt.float32
BF16 = mybir.dt.bfloat16
I16 = mybir.dt.int16

B, S, D = 2, 8192, 1024
H = 16
DEPTH = 64          # head dim
G = 4               # head groups (one per core within a batch)
HPG = 4             # heads per group
DG = HPG * DEPTH    # 256 features per group
QT = 512            # q tile
KC = 128            # k chunk (matmul contraction tile)
NDC = D // 128      # 8 contraction chunks for projections

AFT = mybir.ActivationFunctionType
ALU = mybir.AluOpType

SCALE = 0.125                                  # 1/sqrt(64)
SCH_A = SCALE * np.log2(np.e) * 128.0          # schraudolph multiplier
SCH_B0 = 127.0 * 128.0                         # exponent bias in bf16 bits


def build_program(seq=S, dve_num=1, dve_den=3, boff=-7.4):
    """Build the per-core Bass program. Returns the compiled Bacc object."""
    assert seq % QT == 0
    nqt = seq // QT
    nkc = seq // KC
    nsc = seq // QT
    dt = BF16

    nc = bacc.Bacc("TRN2", target_bir_lowering=False, debug=False,
                   enable_asserts=False, num_devices=8)

    # ---- external I/O ----
    qT = nc.dram_tensor("qT", [D, seq], dt, kind="ExternalInput").ap()
    kT = nc.dram_tensor("kT", [D, seq], dt, kind="ExternalInput").ap()
    vT = nc.dram_tensor("vT", [D, seq], dt, kind="ExternalInput").ap()
    Wq = nc.dram_tensor("Wq", [D, DG], dt, kind="ExternalInput").ap()
    Wk = nc.dram_tensor("Wk", [D, DG], dt, kind="ExternalInput").ap()
    Wv = nc.dram_tensor("Wv", [D, DG], dt, kind="ExternalInput").ap()
    Wo = nc.dram_tensor("Wo", [DG, D], dt, kind="ExternalInput").ap()
    bq = nc.dram_tensor("bq", [DG, 1], F32, kind="ExternalInput").ap()
    bk = nc.dram_tensor("bk", [DG, 1], F32, kind="ExternalInput").ap()
    bv = nc.dram_tensor("bv", [1, DG], dt, kind="ExternalInput").ap()
    Y = nc.dram_tensor("Y", [seq, D], F32, kind="ExternalOutput").ap()

    with tile.TileContext(nc) as tc:
        from contextlib import ExitStack
        ctx = ExitStack()
        with ctx:
            const = ctx.enter_context(tc.tile_pool(name="const", bufs=1))
            res = ctx.enter_context(tc.tile_pool(name="res", bufs=1))
            xin = ctx.enter_context(tc.tile_pool(name="xin", bufs=3))
            ppool = ctx.enter_context(tc.tile_pool(name="ppool", bufs=3))
            epi = ctx.enter_context(tc.tile_pool(name="epi", bufs=4))
            otp = ctx.enter_context(tc.tile_pool(name="otp", bufs=3))
            ypool = ctx.enter_context(tc.tile_pool(name="ypool", bufs=3))
            ps_small = ctx.enter_context(
                tc.tile_pool(name="ps_small", bufs=2, space="PSUM"))
            ps_logit = ctx.enter_context(
                tc.tile_pool(name="ps_logit", bufs=2, space="PSUM"))
            ps_pv = ctx.enter_context(
                tc.tile_pool(name="ps_pv", bufs=1, space="PSUM"))

            # ---- constants ----
            ident = const.tile([128, 128], F32, tag="ident")
            make_identity(nc, ident[:])
            ones_f32 = const.tile([128, 128], F32, tag="ones_f32")
            nc.any.memset(ones_f32[:], 1.0)
            ones_row = const.tile([1, 128], dt, tag="ones_row")
            nc.vector.tensor_copy(ones_row[:], ones_f32[0:1, :])
            ident_bf = const.tile([128, 128], dt, tag="ident_bf")
            nc.vector.tensor_copy(ident_bf[:], ident[:])

            wq_sb = [const.tile([128, DG], dt, tag=f"wq{dc}", name=f"wq{dc}")
                     for dc in range(NDC)]
            wk_sb = [const.tile([128, DG], dt, tag=f"wk{dc}", name=f"wk{dc}")
                     for dc in range(NDC)]
            wv_sb = [const.tile([128, DG], dt, tag=f"wv{dc}", name=f"wv{dc}")
                     for dc in range(NDC)]
            for dc in range(NDC):
                nc.sync.dma_start(wq_sb[dc][:], Wq[ts(dc, 128), :])
                nc.sync.dma_start(wk_sb[dc][:], Wk[ts(dc, 128), :])
                nc.sync.dma_start(wv_sb[dc][:], Wv[ts(dc, 128), :])
            wo_sb = [const.tile([128, D], dt, tag=f"wo{i}", name=f"wo{i}")
                     for i in range(2)]
            for i in range(2):
                nc.sync.dma_start(wo_sb[i][:], Wo[ts(i, 128), :])
            bq_sb = [const.tile([128, 1], F32, tag=f"bq{i}", name=f"bq{i}")
                     for i in range(2)]
            bk_sb = [const.tile([128, 1], F32, tag=f"bk{i}", name=f"bk{i}")
                     for i in range(2)]
            for i in range(2):
                nc.sync.dma_start(bq_sb[i][:], bq[ts(i, 128), :])
                nc.sync.dma_start(bk_sb[i][:], bk[ts(i, 128), :])
            bv_sb = const.tile([1, DG], dt, tag="bv_sb")
            nc.sync.dma_start(bv_sb[:], bv[:, :])

            # ---- resident tensors (persist for the whole kernel) ----
            ktr = [res.tile([128, seq], dt, tag=f"ktr{p}", name=f"ktr{p}")
                   for p in range(2)]
            qtr = [res.tile([128, seq], dt, tag=f"qtr{p}", name=f"qtr{p}")
                   for p in range(2)]
            VP_W = 2 * (DEPTH + 1)  # per-pair per-chunk: 2 heads x [V_h|1]
            vpr = [res.tile([128, nkc, VP_W], dt, tag=f"vpr{p}",
                            name=f"vpr{p}") for p in range(2)]
            ot0 = res.tile([128, seq], dt, tag="ot0", name="ot0")
            # ones columns of V' are constant: write once.
            for p in range(2):
                vh = vpr[p][:].rearrange("p k (h x) -> p k h x", x=DEPTH + 1)
                nc.vector.memset(vh[:, :, :, DEPTH:DEPTH + 1], 1.0)

            # ---- projection emitters ----
            def load_x(src, sc, eng):
                xt = xin.tile([128, NDC, QT], dt, tag="xt", name="xt")
                rr = src.rearrange("(c p) s -> p c s", p=128)
                eng.dma_start(xt[:], rr[:, :, ts(sc, QT)])
                return xt

            def qkproj_emit(sc, xt, w_sb, b_sb, dst, f):
                ps = ps_small.tile([128, QT], F32, tag="ps_proj", name="ps")
                for dc in range(NDC):
                    nc.tensor.matmul(
                        ps[:], w_sb[dc][:, ts(f, 128)], xt[:, dc, :],
                        start=(dc == 0), stop=(dc == NDC - 1))
                nc.vector.tensor_scalar_add(
                    dst[f][:, ts(sc, QT)], ps[:], b_sb[f][:])

            def vproj_emit(sc, xt, sub):
                ps = ps_small.tile([128, DG], F32, tag="ps_proj", name="ps")
                for dc in range(NDC):
                    nc.tensor.matmul(
                        ps[:], xt[:, dc, ts(sub, 128)], wv_sb[dc][:],
                        start=(dc == 0), stop=False)
                nc.tensor.matmul(ps[:], ones_row[:], bv_sb[:],
                                 start=False, stop=True)
                kc_i = sc * (QT // 128) + sub
                for p in range(2):
                    src = ps[:, ds(p * 2 * DEPTH, 2 * DEPTH)].rearrange(
                        "p (h x) -> p h x", x=DEPTH)
                    dstv = vpr[p][:, kc_i, :].rearrange(
                        "p (h x) -> p h x", x=DEPTH + 1)
                    nc.vector.tensor_copy(dstv[:, :, 0:DEPTH], src)

            # ================= P1: K + V projections =================
            for sc in range(nsc):
                xk = load_x(kT, sc, nc.sync)
                for f in range(2):
                    qkproj_emit(sc, xk, wk_sb, bk_sb, ktr, f)
                xv = load_x(vT, sc, nc.gpsimd)
                for sub in range(QT // 128):
                    vproj_emit(sc, xv, sub)
            # first q chunk up front; the rest interleaves into pair 0.
            xq = load_x(qT, 0, nc.sync)
            for f in range(2):
                qkproj_emit(0, xq, wq_sb, bq_sb, qtr, f)

            # ================= P2: attention =================
            # deferred-emission queue: small closures (qproj chunks,
            # epilogue chunks, Wo chunks) drained between slots.
            pending = []

            def drain(n=1):
                for _ in range(n):
                    if pending:
                        pending.pop(0)()

            def make_epi_chunks(pair, qt, u_sbs):
                """normalize U' -> O^T; pair0 -> ot0 resident, pair1 ->
                ot_acc tile; pair1 also appends the Wo chunks for qt."""
                chunks = []
                if pair == 1:
                    ot_acc = otp.tile([128, QT], dt, tag="ot_acc",
                                      name="ot_acc")
                else:
                    ot_acc = None
                for h in range(2):
                    u_sb = u_sbs[h]
                    for blk in range(QT // 128):
                        def c_norm(h=h, blk=blk, u_sb=u_sb):
                            t1 = ps_small.tile([128, DEPTH + 1], F32,
                                               tag="ps_proj", name="t1")
                            nc.tensor.transpose(
                                t1[:], u_sb[:, ts(blk, 128)],
                                ident[0:DEPTH + 1, 0:DEPTH + 1])
                            rq = epi.tile([128, 1], F32, tag="rq")
                            nc.vector.reciprocal(
                                rq[:], t1[:, DEPTH:DEPTH + 1])
                            oq = epi.tile([128, DEPTH], F32, tag="oq")
                            nc.vector.tensor_scalar_mul(
                                oq[:], t1[:, 0:DEPTH], rq[:])
                            t2 = ps_small.tile([DEPTH, 128], F32,
                                               tag="ps_proj", name="t2")
                            nc.tensor.transpose(t2[:], oq[:], ident[:])
                            if pair == 0:
                                dst = ot0[ds(DEPTH * h, DEPTH),
                                          ds(qt * QT + blk * 128, 128)]
                            else:
                                dst = ot_acc[ds(DEPTH * h, DEPTH),
                                             ts(blk, 128)]
                            nc.vector.tensor_copy(dst, t2[:])
                        chunks.append(c_norm)
                if pair == 1:
                    for qs in range(QT // 128):
                        def c_wo(qs=qs, ot_acc=ot_acc, qt=qt):
                            for f in range(2):
                                yp = ps_small.tile([128, 512], F32,
                                                   tag="ps_proj", name="yp")
                                nc.tensor.matmul(
                                    yp[:],
                                    ot0[:, ds(qt * QT + qs * 128, 128)],
                                    wo_sb[0][:, ts(f, 512)],
                                    start=True, stop=False)
                                nc.tensor.matmul(
                                    yp[:], ot_acc[:, ts(qs, 128)],
                                    wo_sb[1][:, ts(f, 512)],
                                    start=False, stop=True)
                                ysb = ypool.tile([128, 512], F32, tag="ysb")
                                nc.vector.tensor_copy(ysb[:], yp[:])
                                nc.gpsimd.dma_start(
                                    Y[ds(qt * QT + qs * 128, 128),
                                      ts(f, 512)], ysb[:])
                        chunks.append(c_wo)
                return chunks

            def make_qproj_chunks(sc):
                chunks = []
                xq = [None]

                def c_load(sc=sc):
                    xq[0] = load_x(qT, sc, nc.sync)
                chunks.append(c_load)
                for f in range(2):
                    def c_proj(sc=sc, f=f):
                        qkproj_emit(sc, xq[0], wq_sb, bq_sb, qtr, f)
                    chunks.append(c_proj)
                return chunks

            sch_b = SCH_B0 + boff

            for pair in range(2):
                ktp, qtp_, vpp = ktr[pair], qtr[pair], vpr[pair]
                for qt in range(nqt):
                    if pair == 0 and qt + 1 < nsc:
                        pending.extend(make_qproj_chunks(qt + 1))
                    pv_ps = [ps_pv.tile([DEPTH + 1, QT], F32,
                                        tag=f"pv{h}", bufs=1, name=f"pv{h}")
                             for h in range(2)]
                    lg_tiles = {}

                    def emit_lg(i, qt=qt, ktp=ktp, qtp_=qtp_,
                                lg_tiles=lg_tiles):
                        lg = ps_logit.tile([128, 2 * QT], F32, tag="lg",
                                           name="lg")
                        for h in range(2):
                            nc.tensor.matmul(
                                lg[:, ts(h, QT)],
                                ktp[ds(DEPTH * h, DEPTH), ts(i, KC)],
                                qtp_[ds(DEPTH * h, DEPTH), ts(qt, QT)],
                                start=True, stop=True,
                                tile_position=(DEPTH * h, 0))
                        lg_tiles[i] = lg

                    # software-pipelined slot loop: one slot = one k-chunk
                    # (both heads); exp(i) then lg(i+2) then pv(i).
                    emit_lg(0)
                    if nkc > 1:
                        emit_lg(1)
                    for i in range(nkc):
                        lg = lg_tiles.pop(i)
                        use_dve = (((i + 1) * dve_num) // dve_den) != (
                            (i * dve_num) // dve_den)
                        if use_dve:
                            pt_i = ppool.tile([128, 2 * QT], I16,
                                              tag="ptD", bufs=2, name="ptD")
                            nc.vector.tensor_scalar(
                                pt_i[:], lg[:], SCH_A, sch_b,
                                op0=ALU.mult, op1=ALU.add)
                            pt = pt_i[:].bitcast(dt)
                        else:
                            pt_b = ppool.tile([128, 2 * QT], dt,
                                              tag="ptA", bufs=3, name="ptA")
                            nc.scalar.activation(
                                pt_b[:], lg[:], AFT.Exp, scale=SCALE)
                            pt = pt_b[:]
                        if i + 2 < nkc:
                            emit_lg(i + 2)
                        for h in range(2):
                            nc.tensor.matmul(
                                pv_ps[h][:],
                                vpp[:, i, ds(h * (DEPTH + 1), DEPTH + 1)],
                                pt[:, ts(h, QT)],
                                start=(i == 0), stop=(i == nkc - 1))
                        if i % 3 == 1:
                            drain(1)

                    # spill U' out of PSUM (frees pv banks), defer the rest
                    u_sbs = []
                    for h in range(2):
                        u_sb = epi.tile([DEPTH + 1, QT], F32, tag="u_sb",
                                        bufs=4)
                        nc.vector.tensor_copy(u_sb[:], pv_ps[h][:])
                        u_sbs.append(u_sb)
                    pending.extend(make_epi_chunks(pair, qt, u_sbs))
            drain(len(pending))
    nc.compile()
    return nc


_NC_CACHE = {}


def _get_program(key_args):
    if key_args not in _NC_CACHE:
        _NC_CACHE[key_args] = build_program(*key_args)
    return _NC_CACHE[key_args]


def make_in_maps(inputs, seq=S):
    """Host-side sharding: per-core input dicts."""
    try:
        import ml_dtypes
        bf16 = ml_dtypes.bfloat16
    except ImportError:
        bf16 = None

    def cast(x):
        return x.astype(bf16)

    q = np.asarray(inputs["q"], np.float32)
    k = np.asarray(inputs["k"], np.float32)
    v = np.asarray(inputs["v"], np.float32)
    Wq = np.asarray(inputs["Wq"], np.float32)
    Wk = np.asarray(inputs["Wk"], np.float32)
    Wv = np.asarray(inputs["Wv"], np.float32)
    Wo = np.asarray(inputs["Wo"], np.float32)
    bq = np.asarray(inputs["bq"], np.float32)
    bk = np.asarray(inputs["bk"], np.float32)
    bv = np.asarray(inputs["bv"], np.float32)

    qTb = [np.ascontiguousarray(q[b].T) for b in range(B)]
    kTb = [np.ascontiguousarray(k[b].T) for b in range(B)]
    vTb = [np.ascontiguousarray(v[b].T) for b in range(B)]

    in_maps = []
    for c in range(8):
        b, g = c // G, c % G
        cols = slice(g * DG, (g + 1) * DG)
        in_maps.append({
            "qT": cast(qTb[b]), "kT": cast(kTb[b]), "vT": cast(vTb[b]),
            "Wq": cast(np.ascontiguousarray(Wq[:, cols])),
            "Wk": cast(np.ascontiguousarray(Wk[:, cols])),
            "Wv": cast(np.ascontiguousarray(Wv[:, cols])),
            "Wo": cast(np.ascontiguousarray(Wo[cols, :])),
            "bq": np.ascontiguousarray(bq[cols].reshape(DG, 1)),
            "bk": np.ascontiguousarray(bk[cols].reshape(DG, 1)),
            "bv": cast(np.ascontiguousarray(bv[cols].reshape(1, DG))),
        })
    return in_maps


LAST_RESULT = None


def kernel(**inputs):
    global LAST_RESULT
    dve_num = int(os.environ.get("MHA_DVE_NUM", "1"))
    dve_den = int(os.environ.get("MHA_DVE_DEN", "3"))
    boff = float(os.environ.get("MHA_BOFF", "-7.4"))
    nc = _get_program((S, dve_num, dve_den, boff))
    in_maps = make_in_maps(inputs, S)
    res = run_bass_kernel_spmd(nc, in_maps, list(range(8)))
    LAST_RESULT = res
    bo = np.asarray(inputs["bo"], np.float32)
    out = np.zeros((B, S, D), np.float32)
    for c in range(8):
        b = c // G
        out[b] += res.results[c]["Y"]
    out += bo[None, None, :]
    return out


if __name__ == "__main__":
    # smoke build
    nc = build_program(1024)
    print("built ok")
